# revision 27
# baseline (speedup 1.0000x reference)
"""Trainium2 Bass kernel for the CAM factorized-attention module.

Reference computation (per batch element b, C=256, N=P*H*W=12288, h=8 heads,
Ch=32):
    x1   = x[b].reshape(C, N).T                      # [N, C]
    qkv  = x1 @ W_qkv + b_qkv                        # [N, 3C]
    q, k, v  (each [h, N, Ch])
    kw   = softmax(k, axis=N)
    kv   = kw^T @ v (per head)                       # [h, Ch, Ch]
    fa   = q @ kv                                    # [h, N, Ch]
    out  = (scale * fa).reshape(N, C) @ W_proj + b_proj
    res  = gamma * out.T.reshape(C, P, H, W) + x[b]

Sharding: data-parallel over B — core i computes batch element i, no
collectives.

Precision plan: the attention branch is ~0.3% of the output magnitude, so it
tolerates aggressive quantization; the residual path needs only bf16 (output
rel err ~3.8e-3 vs the 2e-2 gate, verified both in numpy and on the device).
x ships once as bf16 (residual) and once as fp8e4 (matmul operand); all big
matmuls run fp8 DoubleRow (contraction 256 in one pass, 2 cols/cycle);
E=exp(k) and v are stored fp8e4 so the kv accumulation is DoubleRow too.
The folded map M is cast to fp8e5m2 at NATURAL scale (entries ~1e-4 sit in
e5m2 normal range), which removes the 2^17 descale so the phase-2 epilogue
is a single op per tile.

Algebraic restructuring (exact up to rounding):
  * k bias cancels in softmax -> dropped; no max-subtraction needed (|k|<~5).
  * softmax denominators ride as a ones column in the kv matmul; the
    normalization is applied to the tiny per-head [Ch,Ch] kv matrix.
  * v bias folds into kv; scale & gamma fold into W_proj; gamma into b_proj.
  * the branch collapses to ONE linear map: out = M^T x + be 1^T + x,
    M = Wq kvblk Wp' fused on-chip with 14 tiny matmuls after phase 1.

Schedule (cost-model, ~64us/core): phase 1 iterates 48 pairs of 128-token
chunks: 2 DoubleRow matmuls into a 2-bank PSUM slot (3 slots, so the
write-after-read slack is 3x the dependency loop and the drains run
back-to-back), one exp (ACT, 612ns) and one v-copy (DVE, 658ns) per pair
into an 8-deep ring of fp8 tiles, kv DoubleRow matmuls lagged 3 pairs so
the in-order PE never stalls; the kv accumulator is a single PSUM bank.
The fold shortens its serial chain by host-precomputing the bv term
(M_const, bias) and folding the softmax denominators into scaled copies of
WqT/bq (recip is per-k-partition).  Phase 2 alternates two drain routes per
[128,512] tile: even units add the residual INSIDE PSUM via an identity-
matmul accumulate (PE) so the drain is one ACT bias-pass; odd units use one
DVE scalar_tensor_tensor (pp+be)+xbf.  Output leaves in 4 bf16 slabs, one
DMA per 1024 tokens; phase 2 runs at the out-DMA floor (transfers
back-to-back).  Phase 1 is v-copy-bound (~33us), fold ~4.3us, phase 2
~21us incl the DMA tail.
"""

import sys

sys.path.insert(0, "/opt/trn_rl_repo")

import numpy as np
import ml_dtypes

import concourse.bacc as bacc
import concourse.mybir as mybir
from concourse.tile import TileContext
from concourse.bass_utils import run_bass_kernel_spmd

FP32 = mybir.dt.float32
BF16 = mybir.dt.bfloat16
FP8 = mybir.dt.float8e4
FP8E5 = mybir.dt.float8e5
AF = mybir.ActivationFunctionType
DR = mybir.MatmulPerfMode.DoubleRow

C = 256
N = 12288
NCORES = 8
NCHUNK = N // 128   # 96 chunks of 128 tokens
NGROUP = NCHUNK // 3  # 32 phase-1 groups of 3 chunks
NPAIR = NCHUNK // 2   # 48 DoubleRow token-pairs
NJUMBO = N // 512     # 24 phase-2 chunks of 512 tokens

_CACHE = {}


def _build_nc():
    from concourse.alu_op_type import AluOpType

    nc = bacc.Bacc(trn_type="TRN2", target_bir_lowering=False)

    x8_d = nc.declare_dram_parameter("x8", [128, 2, N], FP8, False)
    xbf_d = nc.declare_dram_parameter("xbf", [2, 128, N], BF16, False)
    wkv8_d = nc.declare_dram_parameter("wkv8", [128, 2, 512], FP8, False)
    # bf16 pack: cols 0:256 WqT, 256:512 Wp', 512:513 bq, 513:641 I128,
    # 641:897 M_const (host-folded bv contribution to M, per kc=t: 2 mt blocks)
    wbf_d = nc.declare_dram_parameter("wbf", [2, 128, 897], BF16, False)
    # fp32 pack: effective proj bias (gamma*b_proj + host-folded bv term)
    wf32_d = nc.declare_dram_parameter("wf32", [2, 128, 1], FP32, False)
    out_d = nc.declare_dram_parameter("out", [128, 2, N], BF16, True)

    with TileContext(nc) as tc:
        with (
            tc.tile_pool(name="const", bufs=1) as const,
            tc.tile_pool(name="resident", bufs=1) as resident,
        ):
            # --- resident tensors -------------------------------------------
            x8 = resident.tile([128, 2, N], FP8, name="x8")
            xbf = [resident.tile([128, N], BF16, name=f"xbf{t}") for t in range(2)]
            osl = [resident.tile([128, 2, 1024], BF16, name=f"osl{s}") for s in range(4)]
            # manual 8-deep ring of per-pair E/v tiles: separate tile
            # objects so the per-tile dependency tracking pipelines
            E3 = [resident.tile([128, 512], FP8, name=f"E3_{j}") for j in range(8)]
            vb3 = [resident.tile([128, 516], FP8, name=f"vb3_{j}") for j in range(8)]
            wkv8 = const.tile([128, 2, 512], FP8, name="wkv8")
            wbf = [const.tile([128, 897], BF16, name=f"wbf{t}") for t in range(2)]
            wf32 = [const.tile([128, 1], FP32, name=f"wf32{t}") for t in range(2)]
            kvsb = const.tile([128, 256], BF16, name="kvsb")
            wqts = [const.tile([128, 256], BF16, name=f"wqts{t}") for t in range(2)]
            bqs = [const.tile([128, 1], BF16, name=f"bqs{t}") for t in range(2)]
            Gp = [
                [const.tile([128, 128], BF16, name=f"Gp{t}{kc}") for kc in range(2)]
                for t in range(2)
            ]
            M8 = [const.tile([128, 2, 128], FP8E5, name=f"M8{mt}") for mt in range(2)]
            cq = [const.tile([128, 1], BF16, name=f"cq{t}") for t in range(2)]
            be = [const.tile([128, 1], FP32, name=f"be{mt}") for mt in range(2)]
            recip = [const.tile([128, 1], FP32, name=f"recip{t}") for t in range(2)]

            # phase-1 gates first: wkv8, then x8 in staggered pieces so the
            # first matmul starts as early as possible
            n0 = 0
            for i, sz in enumerate((256, 256, 512, 1024, 1536, 2048, 3072, 3584)):
                nc.sync.dma_start(x8[:, :, n0 : n0 + sz], x8_d[:, :, n0 : n0 + sz])
                if i == 0:
                    # wkv8 rides the ACT queue so its HWDGE prep overlaps
                    nc.scalar.dma_start(wkv8[:], wkv8_d[:, :, :])
                n0 += sz
            assert n0 == N
            for t in range(2):
                nc.sync.dma_start(wbf[t][:], wbf_d[t])
                nc.sync.dma_start(wf32[t][:], wf32_d[t])
            # ones columns for the softmax denominators
            for j in range(8):
                nc.vector.memset(
                    vb3[j][:].rearrange("p (s t x) -> p s t x", t=2, x=129)[
                        :, :, :, 128:129
                    ],
                    1.0,
                )
            # xbf only matters from phase 2 on; stream it during phase 1
            PIECE = N // 4
            for i in range(4):
                for t in range(2):
                    nc.sync.dma_start(
                        xbf[t][:, i * PIECE : (i + 1) * PIECE],
                        xbf_d[t, :, i * PIECE : (i + 1) * PIECE],
                    )

            wqt = [wbf[t][:, 0:256] for t in range(2)]
            wp = [wbf[t][:, 256:512] for t in range(2)]
            bq = [wbf[t][:, 512:513] for t in range(2)]
            I128 = wbf[0][:, 513:641]
            Mc = [
                [wbf[kc][:, 641 + mt * 128 : 641 + (mt + 1) * 128] for mt in range(2)]
                for kc in range(2)
            ]
            bp = [wf32[t][:, 0:1] for t in range(2)]

            # --- phase 1: k||v, exp, fp8 kv accumulation (DoubleRow) --------
            # 1-pair (256-token) PSUM slots, 3 buffers: the WAR slack
            # (p1-matmul waits the v-copy 3 pairs back) is ~3x the serial
            # dependency loop, so the DVE v-copies run back-to-back.
            with (
                tc.tile_pool(name="p1ps", bufs=1, space="PSUM") as p1ps,
                tc.tile_pool(name="kvp_ps", bufs=3, space="PSUM") as kvp_ps,
            ):
                kvps = p1ps.tile([128, 258], FP32, name="kvps")

                def emit_kv(pi):
                    Ev = E3[pi % 8][:].rearrange("p (s x) -> p s x", x=256)
                    vv = vb3[pi % 8][:].rearrange("p (s t x) -> p s t x", t=2, x=129)
                    for t in range(2):
                        nc.tensor.matmul(
                            kvps[:, t * 129 : t * 129 + 129],
                            lhsT=Ev[:, :, t * 128 : t * 128 + 128],
                            rhs=vv[:, :, t, :],
                            start=(pi == 0), stop=(pi == NPAIR - 1),
                            perf_mode=DR, skip_group_check=True,
                        )

                for pi in range(NPAIR):
                    kvp = kvp_ps.tile([128, 1024], FP32, name="kvp", tag="kvp")
                    for half in range(2):
                        n0 = (pi * 2 + half) * 128
                        nc.tensor.matmul(
                            kvp[:, half * 512 : half * 512 + 512],
                            lhsT=x8[:, :, n0 : n0 + 128], rhs=wkv8[:],
                            start=True, stop=True, perf_mode=DR,
                        )
                    # kv matmuls for the pair finished 3 iterations ago
                    if pi >= 3:
                        emit_kv(pi - 3)
                    kv2 = kvp[:].rearrange("p (s x) -> p s x", x=512)
                    nc.scalar.activation(
                        E3[pi % 8][:].rearrange("p (s x) -> p s x", x=256),
                        kv2[:, :, 0:256],
                        AF.Exp,
                    )
                    nc.vector.tensor_copy(
                        vb3[pi % 8][:].rearrange("p (s t x) -> p s t x", t=2, x=129)[
                            :, :, :, 0:128
                        ],
                        kv2[:, :, 256:512].rearrange("p s (t c) -> p s t c", c=128),
                    )
                for pi in range(NPAIR - 3, NPAIR):
                    emit_kv(pi)

                # --- finalize: recip, raw-kv diag copy, fold recip into
                # scaled copies of WqT/bq (the bv term was folded on host) ---
                nc.vector.reciprocal(recip[0][:], kvps[:, 128:129])
                nc.vector.tensor_copy(
                    kvsb[:].rearrange("p (t x) -> p t x", x=128),
                    kvps[:].rearrange("p (t x) -> p t x", x=129)[:, :, 0:128],
                )
                nc.vector.reciprocal(recip[1][:], kvps[:, 257:258])
                for t in range(2):
                    nc.vector.tensor_scalar_mul(wqts[t][:], wqt[t], recip[t][:])
                    nc.vector.tensor_scalar_mul(bqs[t][:], bq[t], recip[t][:])

            # --- fold: G' = kvn^T Wq^T, M8 = G'^T Wp' + Mc (fp8e5), bias ---
            # kc-major so each M8 block's inputs finish early; PSUM->SBUF
            # copies split across ACT and DVE to halve the serial chain
            with tc.tile_pool(name="gps", bufs=4, space="PSUM") as gps:
                for kc in range(2):
                    for t in range(2):
                        kvt = kvsb[:, t * 128 : t * 128 + 128]
                        g_ps = gps.tile([128, 128], FP32, name=f"gps{t}{kc}", tag="big")
                        nc.tensor.matmul(
                            g_ps[:],
                            lhsT=kvt,
                            rhs=wqts[t][:, kc * 128 : kc * 128 + 128],
                            start=True, stop=True,
                        )
                        ceng = nc.scalar.copy if t == 0 else nc.vector.tensor_copy
                        ceng(Gp[t][kc][:], g_ps[:])
                for t in range(2):
                    cq_ps = gps.tile([128, 1], FP32, name=f"cqps{t}", tag="little")
                    nc.tensor.matmul(
                        cq_ps[:], lhsT=kvsb[:, t * 128 : t * 128 + 128],
                        rhs=bqs[t][:], start=True, stop=True,
                    )
                    nc.scalar.copy(cq[t][:], cq_ps[:])
                for mt in range(2):
                    for kc in range(2):
                        m_ps = gps.tile([128, 128], FP32, name=f"mps{kc}{mt}", tag="big")
                        for t in range(2):
                            nc.tensor.matmul(
                                m_ps[:],
                                lhsT=Gp[t][kc][:],
                                rhs=wp[t][:, mt * 128 : mt * 128 + 128],
                                start=(t == 0), stop=(t == 1),
                            )
                        nc.vector.tensor_add(M8[mt][:, kc, :], m_ps[:], Mc[kc][mt])
                for mt in range(2):
                    be_ps = gps.tile([128, 1], FP32, name=f"beps{mt}", tag="little")
                    for t in range(2):
                        nc.tensor.matmul(
                            be_ps[:],
                            lhsT=wp[t][:, mt * 128 : mt * 128 + 128],
                            rhs=cq[t][:],
                            start=(t == 0), stop=(t == 1),
                        )
                    nc.vector.tensor_add(be[mt][:], be_ps[:], bp[mt])

            # --- phase 2: pp = M8^T x8 (+ I^T xbf);  drain + be + residual --
            # Two single-pass drain routes per [128,512] tile:
            #   even: residual accumulated in PSUM by an identity matmul,
            #         drain = one ACT bias-pass (pp + be -> bf16)
            #   odd:  one DVE STT  osb = (pp + be) + xbf
            with tc.tile_pool(name="pp_ps", bufs=8, space="PSUM") as pp_ps:
                for cj in range(NJUMBO):
                    n0 = cj * 512
                    slab = osl[(cj // 2) % 4]
                    c0 = (cj % 2) * 512
                    for mt in range(2):
                        act_route = (cj * 2 + mt) % 2 == 0
                        pp = pp_ps.tile([128, 512], FP32, name="pp", tag="pp")
                        nc.tensor.matmul(
                            pp[:], lhsT=M8[mt][:], rhs=x8[:, :, n0 : n0 + 512],
                            start=True, stop=not act_route, perf_mode=DR,
                        )
                        if act_route:
                            nc.tensor.matmul(
                                pp[:], lhsT=I128, rhs=xbf[mt][:, n0 : n0 + 512],
                                start=False, stop=True, skip_group_check=True,
                            )
                            nc.scalar.activation(
                                slab[:, mt, c0 : c0 + 512], pp[:],
                                AF.Identity, bias=be[mt][:],
                            )
                        else:
                            nc.vector.scalar_tensor_tensor(
                                slab[:, mt, c0 : c0 + 512],
                                pp[:],
                                be[mt][:],
                                xbf[mt][:, n0 : n0 + 512],
                                op0=AluOpType.add,
                                op1=AluOpType.add,
                            )
                    if cj == NJUMBO - 2:
                        nc.sync.dma_start(
                            out_d[:, :, cj * 512 : cj * 512 + 512], slab[:, :, 0:512]
                        )
                    elif cj == NJUMBO - 1:
                        nc.sync.dma_start(
                            out_d[:, :, cj * 512 : cj * 512 + 512], slab[:, :, 512:1024]
                        )
                    elif cj % 2 == 1:
                        ns = (cj - 1) * 512
                        nc.sync.dma_start(out_d[:, :, ns : ns + 1024], slab[:])
    nc.finalize()
    return nc


def _get_nc():
    if "nc" not in _CACHE:
        _CACHE["nc"] = _build_nc()
    return _CACHE["nc"]


def _prep_in_maps(x, W_qkv, b_qkv, W_proj, b_proj, gamma):
    bf = ml_dtypes.bfloat16
    f8 = ml_dtypes.float8_e4m3
    scale = 32 ** (-0.5)
    g = float(np.asarray(gamma).reshape(-1)[0])

    # fp8 operands use contraction index c = ko*128 + ki -> layout [ki, ko, :]
    Wkv8 = np.ascontiguousarray(
        W_qkv[:, 256:768].reshape(2, 128, 512).swapaxes(0, 1)).astype(f8)
    Wq = W_qkv[:, 0:256]
    WqT = Wq.T.reshape(2, 128, 256)
    Wpf = W_proj * (scale * g)
    Wp = Wpf.reshape(2, 128, 256)
    bq = b_qkv[0:256].reshape(2, 128, 1)
    I2 = np.broadcast_to(np.eye(128, dtype=np.float32), (2, 128, 128))
    # host-folded bv contribution: Bv[k,v] = bv[v] within each 32-wide head
    bv_vec = b_qkv[512:768]
    head_mask = np.kron(np.eye(8, dtype=np.float32), np.ones((32, 32), np.float32))
    Bv = head_mask * bv_vec[None, :]
    M_const = (Wq @ Bv @ Wpf).astype(np.float32)          # [256, 256]
    Mc = M_const.reshape(2, 128, 2, 128).reshape(2, 128, 256)
    wbf = np.ascontiguousarray(
        np.concatenate([WqT, Wp, bq, I2, Mc], axis=2)).astype(bf)
    wf32 = np.ascontiguousarray(
        (g * b_proj + Wpf.T @ (Bv.T @ b_qkv[0:256])).reshape(2, 128, 1)
    ).astype(np.float32)

    in_maps = []
    for b in range(NCORES):
        xb = np.ascontiguousarray(x[b].reshape(C, N))
        xbf = xb.reshape(2, 128, N).astype(bf)
        x8 = np.ascontiguousarray(
            xbf.astype(f8).swapaxes(0, 1))
        in_maps.append(
            {
                "x8": x8,
                "xbf": xbf,
                "wkv8": Wkv8, "wbf": wbf, "wf32": wf32,
            }
        )
    return in_maps


def kernel(x, W_qkv, b_qkv, W_proj, b_proj, gamma, _trace=False, _trace_kwargs=None):
    x = np.asarray(x, dtype=np.float32)
    nc = _get_nc()
    in_maps = _prep_in_maps(
        x,
        np.asarray(W_qkv, np.float32),
        np.asarray(b_qkv, np.float32),
        np.asarray(W_proj, np.float32),
        np.asarray(b_proj, np.float32),
        np.asarray(gamma, np.float32),
    )
    kw = {}
    if _trace:
        kw = {"trace": True, **(_trace_kwargs or {})}
    res = run_bass_kernel_spmd(nc, in_maps, list(range(NCORES)), **kw)
    out = np.stack(
        [
            res.results[b]["out"]
            .astype(np.float32)
            .transpose(1, 0, 2)
            .reshape(C, 3, 64, 64)
            for b in range(NCORES)
        ]
    )
    if _trace:
        return out, res
    return out


# revision 28
# speedup vs baseline: 1.0100x; 1.0100x over previous
"""Trainium2 Bass kernel for the CAM factorized-attention module.

Reference computation (per batch element b, C=256, N=P*H*W=12288, h=8 heads,
Ch=32):
    x1   = x[b].reshape(C, N).T                      # [N, C]
    qkv  = x1 @ W_qkv + b_qkv                        # [N, 3C]
    q, k, v  (each [h, N, Ch])
    kw   = softmax(k, axis=N)
    kv   = kw^T @ v (per head)                       # [h, Ch, Ch]
    fa   = q @ kv                                    # [h, N, Ch]
    out  = (scale * fa).reshape(N, C) @ W_proj + b_proj
    res  = gamma * out.T.reshape(C, P, H, W) + x[b]

Sharding: data-parallel over B — core i computes batch element i, no
collectives.

Precision plan: the attention branch is ~0.3% of the output magnitude, so it
tolerates aggressive quantization; the residual path needs only bf16 (output
rel err ~3.8e-3 vs the 2e-2 gate, verified both in numpy and on the device).
x ships once as bf16 (residual) and once as fp8e4 (matmul operand); all big
matmuls run fp8 DoubleRow (contraction 256 in one pass, 2 cols/cycle);
E=exp(k) and v are stored fp8e4 so the kv accumulation is DoubleRow too.
The folded map M is cast to fp8e5m2 at NATURAL scale (entries ~1e-4 sit in
e5m2 normal range), which removes the 2^17 descale so the phase-2 epilogue
is a single op per tile.

Algebraic restructuring (exact up to rounding):
  * k bias cancels in softmax -> dropped; no max-subtraction needed (|k|<~5).
  * softmax denominators ride as a ones column in the kv matmul; the
    normalization is applied to the tiny per-head [Ch,Ch] kv matrix.
  * v bias folds into kv; scale & gamma fold into W_proj; gamma into b_proj.
  * the branch collapses to ONE linear map: out = M^T x + be 1^T + x,
    M = Wq kvblk Wp' fused on-chip with 14 tiny matmuls after phase 1.

Schedule (cost-model, ~64us/core): phase 1 iterates 48 pairs of 128-token
chunks: 2 DoubleRow matmuls into a 2-bank PSUM slot (3 slots, so the
write-after-read slack is 3x the dependency loop and the drains run
back-to-back), one exp (ACT, 612ns) and one v-copy (DVE, 658ns) per pair
into an 8-deep ring of fp8 tiles, kv DoubleRow matmuls lagged 3 pairs so
the in-order PE never stalls; the kv accumulator is a single PSUM bank.
The fold shortens its serial chain by host-precomputing the bv term
(M_const, bias) and folding the softmax denominators into scaled copies of
WqT/bq (recip is per-k-partition).  Phase 2 alternates two drain routes per
[128,512] tile: even units add the residual INSIDE PSUM via an identity-
matmul accumulate (PE) so the drain is one ACT bias-pass; odd units use one
DVE scalar_tensor_tensor (pp+be)+xbf.  Output leaves in 4 bf16 slabs, one
DMA per 1024 tokens; phase 2 runs at the out-DMA floor (transfers
back-to-back).  Phase 1 is v-copy-bound (~33us), fold ~4.3us, phase 2
~21us incl the DMA tail.
"""

import sys

sys.path.insert(0, "/opt/trn_rl_repo")

import numpy as np
import ml_dtypes

import concourse.bacc as bacc
import concourse.mybir as mybir
from concourse.tile import TileContext
from concourse.bass_utils import run_bass_kernel_spmd

FP32 = mybir.dt.float32
BF16 = mybir.dt.bfloat16
FP8 = mybir.dt.float8e4
FP8E5 = mybir.dt.float8e5
AF = mybir.ActivationFunctionType
DR = mybir.MatmulPerfMode.DoubleRow

C = 256
N = 12288
NCORES = 8
NCHUNK = N // 128   # 96 chunks of 128 tokens
NGROUP = NCHUNK // 3  # 32 phase-1 groups of 3 chunks
NPAIR = NCHUNK // 2   # 48 DoubleRow token-pairs
NJUMBO = N // 512     # 24 phase-2 chunks of 512 tokens

_CACHE = {}


def _build_nc():
    from concourse.alu_op_type import AluOpType

    nc = bacc.Bacc(trn_type="TRN2", target_bir_lowering=False)

    # x8w fuses the k/v weights (cols 0:512) with the fp8 activations so
    # the phase-1-gating data arrives in ONE first DMA
    x8w_d = nc.declare_dram_parameter("x8w", [128, 2, 512 + N], FP8, False)
    xbf_d = nc.declare_dram_parameter("xbf", [2, 128, N], BF16, False)
    # bf16 pack: cols 0:256 WqT, 256:512 Wp', 512:513 bq, 513:641 I128,
    # 641:897 M_const (host-folded bv contribution to M, per kc=t: 2 mt blocks)
    wbf_d = nc.declare_dram_parameter("wbf", [2, 128, 897], BF16, False)
    # fp32 pack: effective proj bias (gamma*b_proj + host-folded bv term)
    wf32_d = nc.declare_dram_parameter("wf32", [2, 128, 1], FP32, False)
    out_d = nc.declare_dram_parameter("out", [128, 2, N], BF16, True)

    with TileContext(nc) as tc:
        with (
            tc.tile_pool(name="const", bufs=1) as const,
            tc.tile_pool(name="resident", bufs=1) as resident,
        ):
            # --- resident tensors -------------------------------------------
            x8w = resident.tile([128, 2, 512 + N], FP8, name="x8w")
            wkv8 = x8w[:, :, 0:512]
            x8 = x8w[:, :, 512:]
            xbf = [resident.tile([128, N], BF16, name=f"xbf{t}") for t in range(2)]
            osl = [resident.tile([128, 2, 1024], BF16, name=f"osl{s}") for s in range(4)]
            # manual 8-deep ring of per-pair E/v tiles: separate tile
            # objects so the per-tile dependency tracking pipelines
            E3 = [resident.tile([128, 512], FP8, name=f"E3_{j}") for j in range(8)]
            vb3 = [resident.tile([128, 516], FP8, name=f"vb3_{j}") for j in range(8)]
            wbf = [const.tile([128, 897], BF16, name=f"wbf{t}") for t in range(2)]
            wf32 = [const.tile([128, 1], FP32, name=f"wf32{t}") for t in range(2)]
            kvsb = const.tile([128, 256], BF16, name="kvsb")
            wqts = [const.tile([128, 256], BF16, name=f"wqts{t}") for t in range(2)]
            bqs = [const.tile([128, 1], BF16, name=f"bqs{t}") for t in range(2)]
            Gp = [
                [const.tile([128, 128], BF16, name=f"Gp{t}{kc}") for kc in range(2)]
                for t in range(2)
            ]
            M8 = [const.tile([128, 2, 128], FP8E5, name=f"M8{mt}") for mt in range(2)]
            cq = [const.tile([128, 1], BF16, name=f"cq{t}") for t in range(2)]
            be = [const.tile([128, 1], FP32, name=f"be{mt}") for mt in range(2)]
            recip = [const.tile([128, 1], FP32, name=f"recip{t}") for t in range(2)]

            # phase-1 gates first: wkv8, then x8 in staggered pieces so the
            # first matmul starts as early as possible
            n0 = 0
            for sz in (768, 256, 512, 1024, 1536, 2048, 3072, 3584):
                nc.sync.dma_start(
                    x8w[:, :, n0 : n0 + sz], x8w_d[:, :, n0 : n0 + sz]
                )
                n0 += sz
            assert n0 == 512 + N
            for t in range(2):
                nc.sync.dma_start(wbf[t][:], wbf_d[t])
                nc.sync.dma_start(wf32[t][:], wf32_d[t])
            # ones columns for the softmax denominators
            for j in range(8):
                nc.vector.memset(
                    vb3[j][:].rearrange("p (s t x) -> p s t x", t=2, x=129)[
                        :, :, :, 128:129
                    ],
                    1.0,
                )
            # xbf only matters from phase 2 on; stream it during phase 1
            PIECE = N // 4
            for i in range(4):
                for t in range(2):
                    nc.sync.dma_start(
                        xbf[t][:, i * PIECE : (i + 1) * PIECE],
                        xbf_d[t, :, i * PIECE : (i + 1) * PIECE],
                    )

            wqt = [wbf[t][:, 0:256] for t in range(2)]
            wp = [wbf[t][:, 256:512] for t in range(2)]
            bq = [wbf[t][:, 512:513] for t in range(2)]
            I128 = wbf[0][:, 513:641]
            Mc = [
                [wbf[kc][:, 641 + mt * 128 : 641 + (mt + 1) * 128] for mt in range(2)]
                for kc in range(2)
            ]
            bp = [wf32[t][:, 0:1] for t in range(2)]

            # --- phase 1: k||v, exp, fp8 kv accumulation (DoubleRow) --------
            # 1-pair (256-token) PSUM slots, 3 buffers: the WAR slack
            # (p1-matmul waits the v-copy 3 pairs back) is ~3x the serial
            # dependency loop, so the DVE v-copies run back-to-back.
            with (
                tc.tile_pool(name="p1ps", bufs=1, space="PSUM") as p1ps,
                tc.tile_pool(name="kvp_ps", bufs=3, space="PSUM") as kvp_ps,
            ):
                kvps = p1ps.tile([128, 258], FP32, name="kvps")

                def emit_kv(pi):
                    Ev = E3[pi % 8][:].rearrange("p (s x) -> p s x", x=256)
                    vv = vb3[pi % 8][:].rearrange("p (s t x) -> p s t x", t=2, x=129)
                    for t in range(2):
                        nc.tensor.matmul(
                            kvps[:, t * 129 : t * 129 + 129],
                            lhsT=Ev[:, :, t * 128 : t * 128 + 128],
                            rhs=vv[:, :, t, :],
                            start=(pi == 0), stop=(pi == NPAIR - 1),
                            perf_mode=DR, skip_group_check=True,
                        )

                for pi in range(NPAIR):
                    kvp = kvp_ps.tile([128, 1024], FP32, name="kvp", tag="kvp")
                    for half in range(2):
                        n0 = (pi * 2 + half) * 128
                        nc.tensor.matmul(
                            kvp[:, half * 512 : half * 512 + 512],
                            lhsT=x8[:, :, n0 : n0 + 128], rhs=wkv8,
                            start=True, stop=True, perf_mode=DR,
                        )
                    # kv matmuls for the pair finished 3 iterations ago
                    if pi >= 3:
                        emit_kv(pi - 3)
                    kv2 = kvp[:].rearrange("p (s x) -> p s x", x=512)
                    nc.scalar.activation(
                        E3[pi % 8][:].rearrange("p (s x) -> p s x", x=256),
                        kv2[:, :, 0:256],
                        AF.Exp,
                    )
                    nc.vector.tensor_copy(
                        vb3[pi % 8][:].rearrange("p (s t x) -> p s t x", t=2, x=129)[
                            :, :, :, 0:128
                        ],
                        kv2[:, :, 256:512].rearrange("p s (t c) -> p s t c", c=128),
                    )
                for pi in range(NPAIR - 3, NPAIR):
                    emit_kv(pi)

                # --- finalize: recip, raw-kv diag copy, fold recip into
                # scaled copies of WqT/bq (the bv term was folded on host) ---
                nc.vector.reciprocal(recip[0][:], kvps[:, 128:129])
                nc.vector.tensor_copy(
                    kvsb[:].rearrange("p (t x) -> p t x", x=128),
                    kvps[:].rearrange("p (t x) -> p t x", x=129)[:, :, 0:128],
                )
                nc.vector.reciprocal(recip[1][:], kvps[:, 257:258])
                for t in range(2):
                    nc.vector.tensor_scalar_mul(wqts[t][:], wqt[t], recip[t][:])
                    nc.vector.tensor_scalar_mul(bqs[t][:], bq[t], recip[t][:])

            # --- fold: G' = kvn^T Wq^T, M8 = G'^T Wp' + Mc (fp8e5), bias ---
            # kc-major so each M8 block's inputs finish early; PSUM->SBUF
            # copies split across ACT and DVE to halve the serial chain
            with tc.tile_pool(name="gps", bufs=4, space="PSUM") as gps:
                for kc in range(2):
                    for t in range(2):
                        kvt = kvsb[:, t * 128 : t * 128 + 128]
                        g_ps = gps.tile([128, 128], FP32, name=f"gps{t}{kc}", tag="big")
                        nc.tensor.matmul(
                            g_ps[:],
                            lhsT=kvt,
                            rhs=wqts[t][:, kc * 128 : kc * 128 + 128],
                            start=True, stop=True,
                        )
                        ceng = nc.scalar.copy if t == 0 else nc.vector.tensor_copy
                        ceng(Gp[t][kc][:], g_ps[:])
                for t in range(2):
                    cq_ps = gps.tile([128, 1], FP32, name=f"cqps{t}", tag="little")
                    nc.tensor.matmul(
                        cq_ps[:], lhsT=kvsb[:, t * 128 : t * 128 + 128],
                        rhs=bqs[t][:], start=True, stop=True,
                    )
                    nc.scalar.copy(cq[t][:], cq_ps[:])
                for mt in range(2):
                    for kc in range(2):
                        m_ps = gps.tile([128, 128], FP32, name=f"mps{kc}{mt}", tag="big")
                        for t in range(2):
                            nc.tensor.matmul(
                                m_ps[:],
                                lhsT=Gp[t][kc][:],
                                rhs=wp[t][:, mt * 128 : mt * 128 + 128],
                                start=(t == 0), stop=(t == 1),
                            )
                        nc.vector.tensor_add(M8[mt][:, kc, :], m_ps[:], Mc[kc][mt])
                for mt in range(2):
                    be_ps = gps.tile([128, 1], FP32, name=f"beps{mt}", tag="little")
                    for t in range(2):
                        nc.tensor.matmul(
                            be_ps[:],
                            lhsT=wp[t][:, mt * 128 : mt * 128 + 128],
                            rhs=cq[t][:],
                            start=(t == 0), stop=(t == 1),
                        )
                    nc.vector.tensor_add(be[mt][:], be_ps[:], bp[mt])

            # --- phase 2: pp = M8^T x8 (+ I^T xbf);  drain + be + residual --
            # Two single-pass drain routes per [128,512] tile:
            #   even: residual accumulated in PSUM by an identity matmul,
            #         drain = one ACT bias-pass (pp + be -> bf16)
            #   odd:  one DVE STT  osb = (pp + be) + xbf
            with tc.tile_pool(name="pp_ps", bufs=8, space="PSUM") as pp_ps:
                for cj in range(NJUMBO):
                    n0 = cj * 512
                    slab = osl[(cj // 2) % 4]
                    c0 = (cj % 2) * 512
                    for mt in range(2):
                        act_route = (cj * 2 + mt) % 2 == 0
                        pp = pp_ps.tile([128, 512], FP32, name="pp", tag="pp")
                        nc.tensor.matmul(
                            pp[:], lhsT=M8[mt][:], rhs=x8[:, :, n0 : n0 + 512],
                            start=True, stop=not act_route, perf_mode=DR,
                        )
                        if act_route:
                            nc.tensor.matmul(
                                pp[:], lhsT=I128, rhs=xbf[mt][:, n0 : n0 + 512],
                                start=False, stop=True, skip_group_check=True,
                            )
                            nc.scalar.activation(
                                slab[:, mt, c0 : c0 + 512], pp[:],
                                AF.Identity, bias=be[mt][:],
                            )
                        else:
                            nc.vector.scalar_tensor_tensor(
                                slab[:, mt, c0 : c0 + 512],
                                pp[:],
                                be[mt][:],
                                xbf[mt][:, n0 : n0 + 512],
                                op0=AluOpType.add,
                                op1=AluOpType.add,
                            )
                    if cj == NJUMBO - 2:
                        nc.sync.dma_start(
                            out_d[:, :, cj * 512 : cj * 512 + 512], slab[:, :, 0:512]
                        )
                    elif cj == NJUMBO - 1:
                        nc.sync.dma_start(
                            out_d[:, :, cj * 512 : cj * 512 + 512], slab[:, :, 512:1024]
                        )
                    elif cj % 2 == 1:
                        ns = (cj - 1) * 512
                        nc.sync.dma_start(out_d[:, :, ns : ns + 1024], slab[:])
    nc.finalize()
    return nc


def _get_nc():
    if "nc" not in _CACHE:
        _CACHE["nc"] = _build_nc()
    return _CACHE["nc"]


def _prep_in_maps(x, W_qkv, b_qkv, W_proj, b_proj, gamma):
    bf = ml_dtypes.bfloat16
    f8 = ml_dtypes.float8_e4m3
    scale = 32 ** (-0.5)
    g = float(np.asarray(gamma).reshape(-1)[0])

    # fp8 operands use contraction index c = ko*128 + ki -> layout [ki, ko, :]
    Wkv8 = np.ascontiguousarray(
        W_qkv[:, 256:768].reshape(2, 128, 512).swapaxes(0, 1)).astype(f8)
    Wq = W_qkv[:, 0:256]
    WqT = Wq.T.reshape(2, 128, 256)
    Wpf = W_proj * (scale * g)
    Wp = Wpf.reshape(2, 128, 256)
    bq = b_qkv[0:256].reshape(2, 128, 1)
    I2 = np.broadcast_to(np.eye(128, dtype=np.float32), (2, 128, 128))
    # host-folded bv contribution: Bv[k,v] = bv[v] within each 32-wide head
    bv_vec = b_qkv[512:768]
    head_mask = np.kron(np.eye(8, dtype=np.float32), np.ones((32, 32), np.float32))
    Bv = head_mask * bv_vec[None, :]
    M_const = (Wq @ Bv @ Wpf).astype(np.float32)          # [256, 256]
    Mc = M_const.reshape(2, 128, 2, 128).reshape(2, 128, 256)
    wbf = np.ascontiguousarray(
        np.concatenate([WqT, Wp, bq, I2, Mc], axis=2)).astype(bf)
    wf32 = np.ascontiguousarray(
        (g * b_proj + Wpf.T @ (Bv.T @ b_qkv[0:256])).reshape(2, 128, 1)
    ).astype(np.float32)

    in_maps = []
    for b in range(NCORES):
        xb = np.ascontiguousarray(x[b].reshape(C, N))
        xbf = xb.reshape(2, 128, N).astype(bf)
        x8 = np.ascontiguousarray(
            xbf.astype(f8).swapaxes(0, 1))
        in_maps.append(
            {
                "x8w": np.concatenate([Wkv8, x8], axis=2),
                "xbf": xbf,
                "wbf": wbf, "wf32": wf32,
            }
        )
    return in_maps


def kernel(x, W_qkv, b_qkv, W_proj, b_proj, gamma, _trace=False, _trace_kwargs=None):
    x = np.asarray(x, dtype=np.float32)
    nc = _get_nc()
    in_maps = _prep_in_maps(
        x,
        np.asarray(W_qkv, np.float32),
        np.asarray(b_qkv, np.float32),
        np.asarray(W_proj, np.float32),
        np.asarray(b_proj, np.float32),
        np.asarray(gamma, np.float32),
    )
    kw = {}
    if _trace:
        kw = {"trace": True, **(_trace_kwargs or {})}
    res = run_bass_kernel_spmd(nc, in_maps, list(range(NCORES)), **kw)
    out = np.stack(
        [
            res.results[b]["out"]
            .astype(np.float32)
            .transpose(1, 0, 2)
            .reshape(C, 3, 64, 64)
            for b in range(NCORES)
        ]
    )
    if _trace:
        return out, res
    return out


# revision 37
# speedup vs baseline: 1.0150x; 1.0050x over previous
"""Trainium2 Bass kernel for the CAM factorized-attention module.

Reference computation (per batch element b, C=256, N=P*H*W=12288, h=8 heads,
Ch=32):
    x1   = x[b].reshape(C, N).T                      # [N, C]
    qkv  = x1 @ W_qkv + b_qkv                        # [N, 3C]
    q, k, v  (each [h, N, Ch])
    kw   = softmax(k, axis=N)
    kv   = kw^T @ v (per head)                       # [h, Ch, Ch]
    fa   = q @ kv                                    # [h, N, Ch]
    out  = (scale * fa).reshape(N, C) @ W_proj + b_proj
    res  = gamma * out.T.reshape(C, P, H, W) + x[b]

Sharding: data-parallel over B — core i computes batch element i, no
collectives.

Precision plan: the attention branch is ~0.3% of the output magnitude, so it
tolerates aggressive quantization; the residual path needs only bf16 (output
rel err ~3.8e-3 vs the 2e-2 gate, verified both in numpy and on the device).
x ships once as bf16 (residual) and once as fp8e4 (matmul operand); all big
matmuls run fp8 DoubleRow (contraction 256 in one pass, 2 cols/cycle);
E=exp(k) and v are stored fp8e4 so the kv accumulation is DoubleRow too.
The folded map M is cast to fp8e5m2 at NATURAL scale (entries ~1e-4 sit in
e5m2 normal range), which removes the 2^17 descale so the phase-2 epilogue
is a single op per tile.

Algebraic restructuring (exact up to rounding):
  * k bias cancels in softmax -> dropped; no max-subtraction needed (|k|<~5).
  * softmax denominators ride as a ones column in the kv matmul; the
    normalization is applied to the tiny per-head [Ch,Ch] kv matrix.
  * v bias folds into kv; scale & gamma fold into W_proj; gamma into b_proj.
  * the branch collapses to ONE linear map: out = M^T x + be 1^T + x,
    M = Wq kvblk Wp' fused on-chip with 14 tiny matmuls after phase 1.

Schedule (cost-model, ~64us/core): phase 1 iterates 48 pairs of 128-token
chunks: 2 DoubleRow matmuls into a 2-bank PSUM slot (3 slots, so the
write-after-read slack is 3x the dependency loop and the drains run
back-to-back), one exp (ACT, 612ns) and one v-copy (DVE, 658ns) per pair
into an 8-deep ring of fp8 tiles, kv DoubleRow matmuls lagged 3 pairs so
the in-order PE never stalls; the kv accumulator is a single PSUM bank.
The fold shortens its serial chain by host-precomputing the bv term
(M_const, bias) and folding the softmax denominators into scaled copies of
WqT/bq (recip is per-k-partition).  Phase 2 alternates two drain routes per
[128,512] tile: even units add the residual INSIDE PSUM via an identity-
matmul accumulate (PE) so the drain is one ACT bias-pass; odd units use one
DVE scalar_tensor_tensor (pp+be)+xbf.  Output leaves in 4 bf16 slabs, one
DMA per 1024 tokens; phase 2 runs at the out-DMA floor (transfers
back-to-back).  Phase 1 is v-copy-bound (~33us), fold ~4.3us, phase 2
~21us incl the DMA tail.
"""

import sys

sys.path.insert(0, "/opt/trn_rl_repo")

import numpy as np
import ml_dtypes

import concourse.bacc as bacc
import concourse.mybir as mybir
from concourse.tile import TileContext
from concourse.bass_utils import run_bass_kernel_spmd

FP32 = mybir.dt.float32
BF16 = mybir.dt.bfloat16
FP8 = mybir.dt.float8e4
FP8E5 = mybir.dt.float8e5
AF = mybir.ActivationFunctionType
DR = mybir.MatmulPerfMode.DoubleRow

C = 256
N = 12288
NCORES = 8
NCHUNK = N // 128   # 96 chunks of 128 tokens
NGROUP = NCHUNK // 3  # 32 phase-1 groups of 3 chunks
NPAIR = NCHUNK // 2   # 48 DoubleRow token-pairs
NJUMBO = N // 512     # 24 phase-2 chunks of 512 tokens

_CACHE = {}


def _build_nc():
    from concourse.alu_op_type import AluOpType

    nc = bacc.Bacc(trn_type="TRN2", target_bir_lowering=False)

    # x8w fuses the k/v weights (cols 0:512) with the fp8 activations so
    # the phase-1-gating data arrives in ONE first DMA
    x8w_d = nc.declare_dram_parameter("x8w", [128, 2, 512 + N], FP8, False)
    xbf_d = nc.declare_dram_parameter("xbf", [2, 128, N], BF16, False)
    # bf16 pack: cols 0:256 WqT, 256:512 Wp', 512:513 bq, 513:641 I128,
    # 641:897 M_const (host-folded bv contribution to M, per kc=t: 2 mt blocks)
    wbf_d = nc.declare_dram_parameter("wbf", [2, 128, 897], BF16, False)
    # fp32 pack: effective proj bias (gamma*b_proj + host-folded bv term)
    wf32_d = nc.declare_dram_parameter("wf32", [2, 128, 1], FP32, False)
    out_d = nc.declare_dram_parameter("out", [128, 2, N], BF16, True)

    with TileContext(nc) as tc:
        with (
            tc.tile_pool(name="const", bufs=1) as const,
            tc.tile_pool(name="resident", bufs=1) as resident,
        ):
            # --- resident tensors -------------------------------------------
            x8w = resident.tile([128, 2, 512 + N], FP8, name="x8w")
            wkv8 = x8w[:, :, 0:512]
            x8 = x8w[:, :, 512:]
            xbf = [resident.tile([128, N], BF16, name=f"xbf{t}") for t in range(2)]
            osl = [resident.tile([128, 2, 1024], BF16, name=f"osl{s}") for s in range(5)]
            # manual 8-deep ring of per-pair E/v tiles: separate tile
            # objects so the per-tile dependency tracking pipelines
            E3 = [resident.tile([128, 512], FP8, name=f"E3_{j}") for j in range(8)]
            vb3 = [resident.tile([128, 516], FP8, name=f"vb3_{j}") for j in range(8)]
            wbf = [const.tile([128, 897], BF16, name=f"wbf{t}") for t in range(2)]
            wf32 = [const.tile([128, 1], FP32, name=f"wf32{t}") for t in range(2)]
            kvsb = const.tile([128, 256], BF16, name="kvsb")
            wqts = [const.tile([128, 256], BF16, name=f"wqts{t}") for t in range(2)]
            bqs = [const.tile([128, 1], BF16, name=f"bqs{t}") for t in range(2)]
            Gp = [
                [const.tile([128, 128], BF16, name=f"Gp{t}{kc}") for kc in range(2)]
                for t in range(2)
            ]
            M8 = [const.tile([128, 2, 128], FP8E5, name=f"M8{mt}") for mt in range(2)]
            cq = [const.tile([128, 1], BF16, name=f"cq{t}") for t in range(2)]
            be = [const.tile([128, 1], FP32, name=f"be{mt}") for mt in range(2)]
            recip = [const.tile([128, 1], FP32, name=f"recip{t}") for t in range(2)]

            # phase-1 gates first: wkv8, then x8 in staggered pieces so the
            # first matmul starts as early as possible
            n0 = 0
            for sz in (768, 256, 512, 1024, 1536, 2048, 3072, 3584):
                nc.sync.dma_start(
                    x8w[:, :, n0 : n0 + sz], x8w_d[:, :, n0 : n0 + sz]
                )
                n0 += sz
            assert n0 == 512 + N
            for t in range(2):
                nc.sync.dma_start(wbf[t][:], wbf_d[t])
                nc.sync.dma_start(wf32[t][:], wf32_d[t])
            # ones columns for the softmax denominators
            for j in range(8):
                nc.vector.memset(
                    vb3[j][:].rearrange("p (s t x) -> p s t x", t=2, x=129)[
                        :, :, :, 128:129
                    ],
                    1.0,
                )
            # xbf only matters from phase 2 on; stream it during phase 1
            PIECE = N // 4
            for i in range(4):
                for t in range(2):
                    nc.sync.dma_start(
                        xbf[t][:, i * PIECE : (i + 1) * PIECE],
                        xbf_d[t, :, i * PIECE : (i + 1) * PIECE],
                    )

            wqt = [wbf[t][:, 0:256] for t in range(2)]
            wp = [wbf[t][:, 256:512] for t in range(2)]
            bq = [wbf[t][:, 512:513] for t in range(2)]
            I128 = wbf[0][:, 513:641]
            Mc = [
                [wbf[kc][:, 641 + mt * 128 : 641 + (mt + 1) * 128] for mt in range(2)]
                for kc in range(2)
            ]
            bp = [wf32[t][:, 0:1] for t in range(2)]

            # --- phase 1: k||v, exp, fp8 kv accumulation (DoubleRow) --------
            # 1-pair (256-token) PSUM slots, 3 buffers: the WAR slack
            # (p1-matmul waits the v-copy 3 pairs back) is ~3x the serial
            # dependency loop, so the DVE v-copies run back-to-back.
            with (
                tc.tile_pool(name="p1ps", bufs=1, space="PSUM") as p1ps,
                tc.tile_pool(name="kvp_ps", bufs=3, space="PSUM") as kvp_ps,
            ):
                kvps = p1ps.tile([128, 258], FP32, name="kvps")

                def emit_kv(pi):
                    Ev = E3[pi % 8][:].rearrange("p (s x) -> p s x", x=256)
                    vv = vb3[pi % 8][:].rearrange("p (s t x) -> p s t x", t=2, x=129)
                    for t in range(2):
                        nc.tensor.matmul(
                            kvps[:, t * 129 : t * 129 + 129],
                            lhsT=Ev[:, :, t * 128 : t * 128 + 128],
                            rhs=vv[:, :, t, :],
                            start=(pi == 0), stop=(pi == NPAIR - 1),
                            perf_mode=DR, skip_group_check=True,
                        )

                for pi in range(NPAIR):
                    kvp = kvp_ps.tile([128, 1024], FP32, name="kvp", tag="kvp")
                    for half in range(2):
                        n0 = (pi * 2 + half) * 128
                        nc.tensor.matmul(
                            kvp[:, half * 512 : half * 512 + 512],
                            lhsT=x8[:, :, n0 : n0 + 128], rhs=wkv8,
                            start=True, stop=True, perf_mode=DR,
                        )
                    # kv matmuls for the pair finished 3 iterations ago
                    if pi >= 3:
                        emit_kv(pi - 3)
                    kv2 = kvp[:].rearrange("p (s x) -> p s x", x=512)
                    nc.scalar.activation(
                        E3[pi % 8][:].rearrange("p (s x) -> p s x", x=256),
                        kv2[:, :, 0:256],
                        AF.Exp,
                    )
                    nc.vector.tensor_copy(
                        vb3[pi % 8][:].rearrange("p (s t x) -> p s t x", t=2, x=129)[
                            :, :, :, 0:128
                        ],
                        kv2[:, :, 256:512].rearrange("p s (t c) -> p s t c", c=128),
                    )
                for pi in range(NPAIR - 3, NPAIR):
                    emit_kv(pi)

                # --- finalize: recip, raw-kv diag copy, fold recip into
                # scaled copies of WqT/bq (the bv term was folded on host) ---
                nc.vector.reciprocal(recip[0][:], kvps[:, 128:129])
                nc.vector.tensor_copy(
                    kvsb[:].rearrange("p (t x) -> p t x", x=128),
                    kvps[:].rearrange("p (t x) -> p t x", x=129)[:, :, 0:128],
                )
                nc.vector.reciprocal(recip[1][:], kvps[:, 257:258])
                for t in range(2):
                    nc.vector.tensor_scalar_mul(wqts[t][:], wqt[t], recip[t][:])
                for t in range(2):
                    nc.vector.tensor_scalar_mul(bqs[t][:], bq[t], recip[t][:])

            # --- fold: G' = kvn^T Wq^T, M8 = G'^T Wp' + Mc (fp8e5), bias ---
            # kc-major so each M8 block's inputs finish early; PSUM->SBUF
            # copies split across ACT and DVE to halve the serial chain
            with tc.tile_pool(name="gps", bufs=4, space="PSUM") as gps:
                for kc in range(2):
                    for t in range(2):
                        kvt = kvsb[:, t * 128 : t * 128 + 128]
                        g_ps = gps.tile([128, 128], FP32, name=f"gps{t}{kc}", tag="big")
                        nc.tensor.matmul(
                            g_ps[:],
                            lhsT=kvt,
                            rhs=wqts[t][:, kc * 128 : kc * 128 + 128],
                            start=True, stop=True,
                        )
                        ceng = nc.scalar.copy if t == 0 else nc.vector.tensor_copy
                        ceng(Gp[t][kc][:], g_ps[:])
                for t in range(2):
                    cq_ps = gps.tile([128, 1], FP32, name=f"cqps{t}", tag="little")
                    nc.tensor.matmul(
                        cq_ps[:], lhsT=kvsb[:, t * 128 : t * 128 + 128],
                        rhs=bqs[t][:], start=True, stop=True,
                    )
                    nc.scalar.copy(cq[t][:], cq_ps[:])
                for mt in range(2):
                    for kc in range(2):
                        act_mc = (mt + kc) % 2 == 1
                        m_ps = gps.tile([128, 128], FP32, name=f"mps{kc}{mt}", tag="big")
                        for t in range(2):
                            nc.tensor.matmul(
                                m_ps[:],
                                lhsT=Gp[t][kc][:],
                                rhs=wp[t][:, mt * 128 : mt * 128 + 128],
                                start=(t == 0), stop=(t == 1 and not act_mc),
                            )
                        if act_mc:
                            # fold Mc in via an identity matmul; ACT drains
                            nc.tensor.matmul(
                                m_ps[:], lhsT=I128, rhs=Mc[kc][mt],
                                start=False, stop=True, skip_group_check=True,
                            )
                            nc.scalar.copy(M8[mt][:, kc, :], m_ps[:])
                        else:
                            nc.vector.tensor_add(M8[mt][:, kc, :], m_ps[:], Mc[kc][mt])
                for mt in range(2):
                    be_ps = gps.tile([128, 1], FP32, name=f"beps{mt}", tag="little")
                    for t in range(2):
                        nc.tensor.matmul(
                            be_ps[:],
                            lhsT=wp[t][:, mt * 128 : mt * 128 + 128],
                            rhs=cq[t][:],
                            start=(t == 0), stop=(t == 1),
                        )
                    nc.vector.tensor_add(be[mt][:], be_ps[:], bp[mt])

            # --- phase 2: pp = M8^T x8 (+ I^T xbf);  drain + be + residual --
            # Two single-pass drain routes per [128,512] tile:
            #   even: residual accumulated in PSUM by an identity matmul,
            #         drain = one ACT bias-pass (pp + be -> bf16)
            #   odd:  one DVE STT  osb = (pp + be) + xbf
            with tc.tile_pool(name="pp_ps", bufs=8, space="PSUM") as pp_ps:
                for cj in range(NJUMBO):
                    n0 = cj * 512
                    # cj0 ships alone (earlier first DMA: the saturated out-
                    # DMA train then finishes earlier); cj23 reuses slab 0
                    if cj <= 1:
                        slab, c0 = osl[0], cj * 512
                    else:
                        slab = osl[1 + ((cj - 2) // 2) % 4]
                        c0 = (cj % 2) * 512
                    for mt in range(2):
                        act_route = (cj * 2 + mt) % 2 == 0
                        pp = pp_ps.tile([128, 512], FP32, name="pp", tag="pp")
                        nc.tensor.matmul(
                            pp[:], lhsT=M8[mt][:], rhs=x8[:, :, n0 : n0 + 512],
                            start=True, stop=not act_route, perf_mode=DR,
                        )
                        if act_route:
                            nc.tensor.matmul(
                                pp[:], lhsT=I128, rhs=xbf[mt][:, n0 : n0 + 512],
                                start=False, stop=True, skip_group_check=True,
                            )
                            nc.scalar.activation(
                                slab[:, mt, c0 : c0 + 512], pp[:],
                                AF.Identity, bias=be[mt][:],
                            )
                        else:
                            nc.vector.scalar_tensor_tensor(
                                slab[:, mt, c0 : c0 + 512],
                                pp[:],
                                be[mt][:],
                                xbf[mt][:, n0 : n0 + 512],
                                op0=AluOpType.add,
                                op1=AluOpType.add,
                            )
                    if cj <= 1:
                        nc.sync.dma_start(
                            out_d[:, :, n0 : n0 + 512], osl[0][:, :, c0 : c0 + 512]
                        )
                    elif cj % 2 == 1:
                        ns = (cj - 1) * 512
                        nc.sync.dma_start(out_d[:, :, ns : ns + 1024], slab[:])
    nc.finalize()
    return nc


def _get_nc():
    if "nc" not in _CACHE:
        _CACHE["nc"] = _build_nc()
    return _CACHE["nc"]


def _prep_in_maps(x, W_qkv, b_qkv, W_proj, b_proj, gamma):
    bf = ml_dtypes.bfloat16
    f8 = ml_dtypes.float8_e4m3
    scale = 32 ** (-0.5)
    g = float(np.asarray(gamma).reshape(-1)[0])

    # fp8 operands use contraction index c = ko*128 + ki -> layout [ki, ko, :]
    Wkv8 = np.ascontiguousarray(
        W_qkv[:, 256:768].reshape(2, 128, 512).swapaxes(0, 1)).astype(f8)
    Wq = W_qkv[:, 0:256]
    WqT = Wq.T.reshape(2, 128, 256)
    Wpf = W_proj * (scale * g)
    Wp = Wpf.reshape(2, 128, 256)
    bq = b_qkv[0:256].reshape(2, 128, 1)
    I2 = np.broadcast_to(np.eye(128, dtype=np.float32), (2, 128, 128))
    # host-folded bv contribution: Bv[k,v] = bv[v] within each 32-wide head
    bv_vec = b_qkv[512:768]
    head_mask = np.kron(np.eye(8, dtype=np.float32), np.ones((32, 32), np.float32))
    Bv = head_mask * bv_vec[None, :]
    M_const = (Wq @ Bv @ Wpf).astype(np.float32)          # [256, 256]
    Mc = M_const.reshape(2, 128, 2, 128).reshape(2, 128, 256)
    wbf = np.ascontiguousarray(
        np.concatenate([WqT, Wp, bq, I2, Mc], axis=2)).astype(bf)
    wf32 = np.ascontiguousarray(
        (g * b_proj + Wpf.T @ (Bv.T @ b_qkv[0:256])).reshape(2, 128, 1)
    ).astype(np.float32)

    in_maps = []
    for b in range(NCORES):
        xb = np.ascontiguousarray(x[b].reshape(C, N))
        xbf = xb.reshape(2, 128, N).astype(bf)
        x8 = np.ascontiguousarray(
            xbf.astype(f8).swapaxes(0, 1))
        in_maps.append(
            {
                "x8w": np.concatenate([Wkv8, x8], axis=2),
                "xbf": xbf,
                "wbf": wbf, "wf32": wf32,
            }
        )
    return in_maps


def kernel(x, W_qkv, b_qkv, W_proj, b_proj, gamma, _trace=False, _trace_kwargs=None):
    x = np.asarray(x, dtype=np.float32)
    nc = _get_nc()
    in_maps = _prep_in_maps(
        x,
        np.asarray(W_qkv, np.float32),
        np.asarray(b_qkv, np.float32),
        np.asarray(W_proj, np.float32),
        np.asarray(b_proj, np.float32),
        np.asarray(gamma, np.float32),
    )
    kw = {}
    if _trace:
        kw = {"trace": True, **(_trace_kwargs or {})}
    res = run_bass_kernel_spmd(nc, in_maps, list(range(NCORES)), **kw)
    out = np.stack(
        [
            res.results[b]["out"]
            .astype(np.float32)
            .transpose(1, 0, 2)
            .reshape(C, 3, 64, 64)
            for b in range(NCORES)
        ]
    )
    if _trace:
        return out, res
    return out


# revision 44
# speedup vs baseline: 1.0156x; 1.0006x over previous
"""Trainium2 Bass kernel for the CAM factorized-attention module.

Reference computation (per batch element b, C=256, N=P*H*W=12288, h=8 heads,
Ch=32):
    x1   = x[b].reshape(C, N).T                      # [N, C]
    qkv  = x1 @ W_qkv + b_qkv                        # [N, 3C]
    q, k, v  (each [h, N, Ch])
    kw   = softmax(k, axis=N)
    kv   = kw^T @ v (per head)                       # [h, Ch, Ch]
    fa   = q @ kv                                    # [h, N, Ch]
    out  = (scale * fa).reshape(N, C) @ W_proj + b_proj
    res  = gamma * out.T.reshape(C, P, H, W) + x[b]

Sharding: data-parallel over B — core i computes batch element i, no
collectives.

Precision plan: the attention branch is ~0.3% of the output magnitude, so it
tolerates aggressive quantization; the residual path needs only bf16 (output
rel err ~3.8e-3 vs the 2e-2 gate, verified both in numpy and on the device).
x ships once as bf16 (residual) and once as fp8e4 (matmul operand); all big
matmuls run fp8 DoubleRow (contraction 256 in one pass, 2 cols/cycle);
E=exp(k) and v are stored fp8e4 so the kv accumulation is DoubleRow too.
The folded map M is cast to fp8e5m2 at NATURAL scale (entries ~1e-4 sit in
e5m2 normal range), which removes the 2^17 descale so the phase-2 epilogue
is a single op per tile.

Algebraic restructuring (exact up to rounding):
  * k bias cancels in softmax -> dropped; no max-subtraction needed (|k|<~5).
  * softmax denominators ride as a ones column in the kv matmul; the
    normalization is applied to the tiny per-head [Ch,Ch] kv matrix.
  * v bias folds into kv; scale & gamma fold into W_proj; gamma into b_proj.
  * the branch collapses to ONE linear map: out = M^T x + be 1^T + x,
    M = Wq kvblk Wp' fused on-chip with 14 tiny matmuls after phase 1.

Schedule (cost-model, ~63.3us/core): phase 1 iterates 48 pairs of 128-token
chunks: 2 DoubleRow matmuls into a 2-bank PSUM slot (3 slots, so the
write-after-read slack is 3x the dependency loop and the drains run
back-to-back), one exp (ACT, 612ns) and one v-copy (DVE, 658ns) per pair
into an 8-deep ring of fp8 tiles, kv DoubleRow matmuls lagged 3 pairs so
the in-order PE never stalls; the kv accumulator is a single PSUM bank.
The fold shortens its serial chain by host-precomputing the bv term
(M_const, bias) and folding the softmax denominators into scaled copies of
WqT/bq (recip is per-k-partition).  Phase 2 alternates two drain routes per
[128,512] tile: even units add the residual INSIDE PSUM via an identity-
matmul accumulate (PE) so the drain is one ACT bias-pass; odd units use one
DVE scalar_tensor_tensor (pp+be)+xbf.  The first two 512-token chunks
ship individually so the out-DMA train starts early; the rest leave in
1024-token bf16 slabs (5 slabs, 4 rotating).  Phase 2 runs with both the
drains and the out-DMA train saturated (~18.6us); phase 1 is v-copy-bound
(~33us); fold ~4us.  The 4 M8 blocks are produced half by DVE adds of the
host Mc constant and half by PE identity-matmuls of Mc with ACT copies, so
the fold tail is parallel across engines.
"""

import sys

sys.path.insert(0, "/opt/trn_rl_repo")

import numpy as np
import ml_dtypes

import concourse.bacc as bacc
import concourse.mybir as mybir
from concourse.tile import TileContext
from concourse.bass_utils import run_bass_kernel_spmd

FP32 = mybir.dt.float32
BF16 = mybir.dt.bfloat16
FP8 = mybir.dt.float8e4
FP8E5 = mybir.dt.float8e5
AF = mybir.ActivationFunctionType
DR = mybir.MatmulPerfMode.DoubleRow

C = 256
N = 12288
NCORES = 8
NCHUNK = N // 128   # 96 chunks of 128 tokens
NGROUP = NCHUNK // 3  # 32 phase-1 groups of 3 chunks
NPAIR = NCHUNK // 2   # 48 DoubleRow token-pairs
NJUMBO = N // 512     # 24 phase-2 chunks of 512 tokens

_CACHE = {}


def _build_nc():
    from concourse.alu_op_type import AluOpType

    nc = bacc.Bacc(trn_type="TRN2", target_bir_lowering=False)

    # x8w fuses the k/v weights (cols 0:512) with the fp8 activations so
    # the phase-1-gating data arrives in ONE first DMA
    x8w_d = nc.declare_dram_parameter("x8w", [128, 2, 512 + N], FP8, False)
    xbf_d = nc.declare_dram_parameter("xbf", [2, 128, N], BF16, False)
    # bf16 pack: cols 0:256 WqT, 256:512 Wp', 512:513 bq, 513:641 I128,
    # 641:897 M_const (host-folded bv contribution to M, per kc=t: 2 mt blocks)
    wbf_d = nc.declare_dram_parameter("wbf", [2, 128, 897], BF16, False)
    # fp32 pack: effective proj bias (gamma*b_proj + host-folded bv term)
    wf32_d = nc.declare_dram_parameter("wf32", [2, 128, 1], FP32, False)
    out_d = nc.declare_dram_parameter("out", [128, 2, N], BF16, True)

    with TileContext(nc) as tc:
        with (
            tc.tile_pool(name="const", bufs=1) as const,
            tc.tile_pool(name="resident", bufs=1) as resident,
        ):
            # --- resident tensors -------------------------------------------
            x8w = resident.tile([128, 2, 512 + N], FP8, name="x8w")
            wkv8 = x8w[:, :, 0:512]
            x8 = x8w[:, :, 512:]
            xbf = [resident.tile([128, N], BF16, name=f"xbf{t}") for t in range(2)]
            osl = [resident.tile([128, 2, 1024], BF16, name=f"osl{s}") for s in range(6)]
            # manual 8-deep ring of per-pair E/v tiles: separate tile
            # objects so the per-tile dependency tracking pipelines
            E3 = [resident.tile([128, 512], FP8, name=f"E3_{j}") for j in range(8)]
            vb3 = [resident.tile([128, 516], FP8, name=f"vb3_{j}") for j in range(8)]
            wbf = [const.tile([128, 897], BF16, name=f"wbf{t}") for t in range(2)]
            wf32 = [const.tile([128, 1], FP32, name=f"wf32{t}") for t in range(2)]
            kvsb = const.tile([128, 256], BF16, name="kvsb")
            wqts = [const.tile([128, 256], BF16, name=f"wqts{t}") for t in range(2)]
            bqs = [const.tile([128, 1], BF16, name=f"bqs{t}") for t in range(2)]
            Gp = [
                [const.tile([128, 128], BF16, name=f"Gp{t}{kc}") for kc in range(2)]
                for t in range(2)
            ]
            M8 = [const.tile([128, 2, 128], FP8E5, name=f"M8{mt}") for mt in range(2)]
            cq = [const.tile([128, 1], BF16, name=f"cq{t}") for t in range(2)]
            be = [const.tile([128, 1], FP32, name=f"be{mt}") for mt in range(2)]
            recip = [const.tile([128, 1], FP32, name=f"recip{t}") for t in range(2)]

            # phase-1 gates first: wkv8, then x8 in staggered pieces so the
            # first matmul starts as early as possible
            n0 = 0
            for sz in (768, 256, 512, 1024, 1536, 2048, 3072, 3584):
                nc.sync.dma_start(
                    x8w[:, :, n0 : n0 + sz], x8w_d[:, :, n0 : n0 + sz]
                )
                n0 += sz
            assert n0 == 512 + N
            for t in range(2):
                nc.sync.dma_start(wbf[t][:], wbf_d[t])
                nc.sync.dma_start(wf32[t][:], wf32_d[t])
            # ones columns for the softmax denominators
            for j in range(8):
                nc.vector.memset(
                    vb3[j][:].rearrange("p (s t x) -> p s t x", t=2, x=129)[
                        :, :, :, 128:129
                    ],
                    1.0,
                )
            # xbf only matters from phase 2 on; stream it during phase 1
            PIECE = N // 4
            for i in range(4):
                for t in range(2):
                    nc.sync.dma_start(
                        xbf[t][:, i * PIECE : (i + 1) * PIECE],
                        xbf_d[t, :, i * PIECE : (i + 1) * PIECE],
                    )

            wqt = [wbf[t][:, 0:256] for t in range(2)]
            wp = [wbf[t][:, 256:512] for t in range(2)]
            bq = [wbf[t][:, 512:513] for t in range(2)]
            I128 = wbf[0][:, 513:641]
            Mc = [
                [wbf[kc][:, 641 + mt * 128 : 641 + (mt + 1) * 128] for mt in range(2)]
                for kc in range(2)
            ]
            bp = [wf32[t][:, 0:1] for t in range(2)]

            # --- phase 1: k||v, exp, fp8 kv accumulation (DoubleRow) --------
            # 1-pair (256-token) PSUM slots, 3 buffers: the WAR slack
            # (p1-matmul waits the v-copy 3 pairs back) is ~3x the serial
            # dependency loop, so the DVE v-copies run back-to-back.
            with (
                tc.tile_pool(name="p1ps", bufs=1, space="PSUM") as p1ps,
                tc.tile_pool(name="kvp_ps", bufs=3, space="PSUM") as kvp_ps,
            ):
                kvps = p1ps.tile([128, 258], FP32, name="kvps")

                def emit_kv(pi):
                    Ev = E3[pi % 8][:].rearrange("p (s x) -> p s x", x=256)
                    vv = vb3[pi % 8][:].rearrange("p (s t x) -> p s t x", t=2, x=129)
                    for t in range(2):
                        nc.tensor.matmul(
                            kvps[:, t * 129 : t * 129 + 129],
                            lhsT=Ev[:, :, t * 128 : t * 128 + 128],
                            rhs=vv[:, :, t, :],
                            start=(pi == 0), stop=(pi == NPAIR - 1),
                            perf_mode=DR, skip_group_check=True,
                        )

                for pi in range(NPAIR):
                    kvp = kvp_ps.tile([128, 1024], FP32, name="kvp", tag="kvp")
                    for half in range(2):
                        n0 = (pi * 2 + half) * 128
                        nc.tensor.matmul(
                            kvp[:, half * 512 : half * 512 + 512],
                            lhsT=x8[:, :, n0 : n0 + 128], rhs=wkv8,
                            start=True, stop=True, perf_mode=DR,
                        )
                    # kv matmuls for the pair finished 3 iterations ago
                    if pi >= 3:
                        emit_kv(pi - 3)
                    kv2 = kvp[:].rearrange("p (s x) -> p s x", x=512)
                    nc.scalar.activation(
                        E3[pi % 8][:].rearrange("p (s x) -> p s x", x=256),
                        kv2[:, :, 0:256],
                        AF.Exp,
                    )
                    nc.vector.tensor_copy(
                        vb3[pi % 8][:].rearrange("p (s t x) -> p s t x", t=2, x=129)[
                            :, :, :, 0:128
                        ],
                        kv2[:, :, 256:512].rearrange("p s (t c) -> p s t c", c=128),
                    )
                for pi in range(NPAIR - 3, NPAIR):
                    emit_kv(pi)

                # --- finalize: recip, raw-kv diag copy, fold recip into
                # scaled copies of WqT/bq (the bv term was folded on host) ---
                nc.vector.reciprocal(recip[0][:], kvps[:, 128:129])
                nc.vector.tensor_copy(
                    kvsb[:].rearrange("p (t x) -> p t x", x=128),
                    kvps[:].rearrange("p (t x) -> p t x", x=129)[:, :, 0:128],
                )
                nc.vector.reciprocal(recip[1][:], kvps[:, 257:258])
                for t in range(2):
                    nc.vector.tensor_scalar_mul(wqts[t][:], wqt[t], recip[t][:])
                for t in range(2):
                    nc.vector.tensor_scalar_mul(bqs[t][:], bq[t], recip[t][:])

            # --- fold: G' = kvn^T Wq^T, M8 = G'^T Wp' + Mc (fp8e5), bias ---
            # kc-major so each M8 block's inputs finish early; PSUM->SBUF
            # copies split across ACT and DVE to halve the serial chain
            with tc.tile_pool(name="gps", bufs=4, space="PSUM") as gps:
                for kc in range(2):
                    for t in range(2):
                        kvt = kvsb[:, t * 128 : t * 128 + 128]
                        g_ps = gps.tile([128, 128], FP32, name=f"gps{t}{kc}", tag="big")
                        nc.tensor.matmul(
                            g_ps[:],
                            lhsT=kvt,
                            rhs=wqts[t][:, kc * 128 : kc * 128 + 128],
                            start=True, stop=True,
                        )
                        ceng = nc.scalar.copy if t == 0 else nc.vector.tensor_copy
                        ceng(Gp[t][kc][:], g_ps[:])
                for t in range(2):
                    cq_ps = gps.tile([128, 1], FP32, name=f"cqps{t}", tag="little")
                    nc.tensor.matmul(
                        cq_ps[:], lhsT=kvsb[:, t * 128 : t * 128 + 128],
                        rhs=bqs[t][:], start=True, stop=True,
                    )
                    nc.scalar.copy(cq[t][:], cq_ps[:])
                for mt in range(2):
                    for kc in range(2):
                        act_mc = (mt + kc) % 2 == 1
                        m_ps = gps.tile([128, 128], FP32, name=f"mps{kc}{mt}", tag="big")
                        for t in range(2):
                            nc.tensor.matmul(
                                m_ps[:],
                                lhsT=Gp[t][kc][:],
                                rhs=wp[t][:, mt * 128 : mt * 128 + 128],
                                start=(t == 0), stop=(t == 1 and not act_mc),
                            )
                        if act_mc:
                            # fold Mc in via an identity matmul; ACT drains
                            nc.tensor.matmul(
                                m_ps[:], lhsT=I128, rhs=Mc[kc][mt],
                                start=False, stop=True, skip_group_check=True,
                            )
                            nc.scalar.copy(M8[mt][:, kc, :], m_ps[:])
                        else:
                            nc.vector.tensor_add(M8[mt][:, kc, :], m_ps[:], Mc[kc][mt])
                for mt in range(2):
                    be_ps = gps.tile([128, 1], FP32, name=f"beps{mt}", tag="little")
                    for t in range(2):
                        nc.tensor.matmul(
                            be_ps[:],
                            lhsT=wp[t][:, mt * 128 : mt * 128 + 128],
                            rhs=cq[t][:],
                            start=(t == 0), stop=(t == 1),
                        )
                    nc.vector.tensor_add(be[mt][:], be_ps[:], bp[mt])

            # --- phase 2: pp = M8^T x8 (+ I^T xbf);  drain + be + residual --
            # Two single-pass drain routes per [128,512] tile:
            #   even: residual accumulated in PSUM by an identity matmul,
            #         drain = one ACT bias-pass (pp + be -> bf16)
            #   odd:  one DVE STT  osb = (pp + be) + xbf
            with tc.tile_pool(name="pp_ps", bufs=8, space="PSUM") as pp_ps:
                for cj in range(NJUMBO):
                    n0 = cj * 512
                    # cj0 ships alone (earlier first DMA: the saturated out-
                    # DMA train then finishes earlier); cj23 reuses slab 0
                    if cj <= 1:
                        slab, c0 = osl[0], cj * 512
                    elif cj <= 3 or cj >= NJUMBO - 2:
                        slab, c0 = osl[5], (cj % 2) * 512
                    else:
                        slab = osl[1 + ((cj - 4) // 2) % 4]
                        c0 = (cj % 2) * 512
                    for mt in range(2):
                        act_route = (cj * 2 + mt) % 2 == 0
                        pp = pp_ps.tile([128, 512], FP32, name="pp", tag="pp")
                        nc.tensor.matmul(
                            pp[:], lhsT=M8[mt][:], rhs=x8[:, :, n0 : n0 + 512],
                            start=True, stop=not act_route, perf_mode=DR,
                        )
                        if act_route:
                            nc.tensor.matmul(
                                pp[:], lhsT=I128, rhs=xbf[mt][:, n0 : n0 + 512],
                                start=False, stop=True, skip_group_check=True,
                            )
                            nc.scalar.activation(
                                slab[:, mt, c0 : c0 + 512], pp[:],
                                AF.Identity, bias=be[mt][:],
                            )
                        else:
                            nc.vector.scalar_tensor_tensor(
                                slab[:, mt, c0 : c0 + 512],
                                pp[:],
                                be[mt][:],
                                xbf[mt][:, n0 : n0 + 512],
                                op0=AluOpType.add,
                                op1=AluOpType.add,
                            )
                    if cj <= 3 or cj >= NJUMBO - 2:
                        nc.sync.dma_start(
                            out_d[:, :, n0 : n0 + 512], slab[:, :, c0 : c0 + 512]
                        )
                    elif cj % 2 == 1:
                        ns = (cj - 1) * 512
                        nc.sync.dma_start(out_d[:, :, ns : ns + 1024], slab[:])
    nc.finalize()
    return nc


def _get_nc():
    if "nc" not in _CACHE:
        _CACHE["nc"] = _build_nc()
    return _CACHE["nc"]


def _prep_in_maps(x, W_qkv, b_qkv, W_proj, b_proj, gamma):
    bf = ml_dtypes.bfloat16
    f8 = ml_dtypes.float8_e4m3
    scale = 32 ** (-0.5)
    g = float(np.asarray(gamma).reshape(-1)[0])

    # fp8 operands use contraction index c = ko*128 + ki -> layout [ki, ko, :]
    Wkv8 = np.ascontiguousarray(
        W_qkv[:, 256:768].reshape(2, 128, 512).swapaxes(0, 1)).astype(f8)
    Wq = W_qkv[:, 0:256]
    WqT = Wq.T.reshape(2, 128, 256)
    Wpf = W_proj * (scale * g)
    Wp = Wpf.reshape(2, 128, 256)
    bq = b_qkv[0:256].reshape(2, 128, 1)
    I2 = np.broadcast_to(np.eye(128, dtype=np.float32), (2, 128, 128))
    # host-folded bv contribution: Bv[k,v] = bv[v] within each 32-wide head
    bv_vec = b_qkv[512:768]
    head_mask = np.kron(np.eye(8, dtype=np.float32), np.ones((32, 32), np.float32))
    Bv = head_mask * bv_vec[None, :]
    M_const = (Wq @ Bv @ Wpf).astype(np.float32)          # [256, 256]
    Mc = M_const.reshape(2, 128, 2, 128).reshape(2, 128, 256)
    wbf = np.ascontiguousarray(
        np.concatenate([WqT, Wp, bq, I2, Mc], axis=2)).astype(bf)
    wf32 = np.ascontiguousarray(
        (g * b_proj + Wpf.T @ (Bv.T @ b_qkv[0:256])).reshape(2, 128, 1)
    ).astype(np.float32)

    in_maps = []
    for b in range(NCORES):
        xb = np.ascontiguousarray(x[b].reshape(C, N))
        xbf = xb.reshape(2, 128, N).astype(bf)
        x8 = np.ascontiguousarray(
            xbf.astype(f8).swapaxes(0, 1))
        in_maps.append(
            {
                "x8w": np.concatenate([Wkv8, x8], axis=2),
                "xbf": xbf,
                "wbf": wbf, "wf32": wf32,
            }
        )
    return in_maps


def kernel(x, W_qkv, b_qkv, W_proj, b_proj, gamma, _trace=False, _trace_kwargs=None):
    x = np.asarray(x, dtype=np.float32)
    nc = _get_nc()
    in_maps = _prep_in_maps(
        x,
        np.asarray(W_qkv, np.float32),
        np.asarray(b_qkv, np.float32),
        np.asarray(W_proj, np.float32),
        np.asarray(b_proj, np.float32),
        np.asarray(gamma, np.float32),
    )
    kw = {}
    if _trace:
        kw = {"trace": True, **(_trace_kwargs or {})}
    res = run_bass_kernel_spmd(nc, in_maps, list(range(NCORES)), **kw)
    out = np.stack(
        [
            res.results[b]["out"]
            .astype(np.float32)
            .transpose(1, 0, 2)
            .reshape(C, 3, 64, 64)
            for b in range(NCORES)
        ]
    )
    if _trace:
        return out, res
    return out
